# revision 5
# baseline (speedup 1.0000x reference)
"""CentroidHead kernel for 8x Trainium2 NeuronCores (Bass/Tile, SPMD).

Device (8 cores, data-parallel over voxels): the FeatDecoder MLP
  h = relu(feats @ W1 + b1); logit = h @ W2; score = sigmoid(logit + b2)
which is the memory/compute-bound part (256 MB feats, ~4.2 GFLOP).

Host: masking, candidate-restricted submanifold 3x3x3 NMS with a
certified top-K filter (any voxel outside the examined top-K set has a
strictly smaller score than the 128th selected peak, so the selection
is provably identical to the full computation), and top-k assembly with
jax.lax.top_k tie semantics (stable, lowest index first).

The matmul packs two 128-voxel chunks into one K=128 weight load
(lhsT = [feats_A^T ; feats_B^T], rhs = [[W1, 0], [0, W1]]), so the PE
processes 256 voxels per 128-column weight load.
"""

import sys

for _p in ("/opt/trn_rl_repo", "/root/.axon_site/_ro/trn_rl_repo"):
    if _p not in sys.path:
        sys.path.append(_p)

import numpy as np

import concourse.bass as bass
import concourse.mybir as mybir
import concourse.tile as tile
import concourse.bacc as bacc
from concourse import bass_utils

# ---- problem constants (hardcoded per spec) ----
N = 1_000_000
B, X, Y, Z = 2, 512, 512, 64
F_IN, H = 64, 32
SCORE_T = np.float32(0.1)
CENT_T = np.float32(0.2)
MAX_PEAKS = 128  # MAX_TREES * B
NCORES = 8

NPER = N // NCORES            # 125000 voxels per core
NCOL = 978                    # SBUF columns; NCOL*128 = 125184 slots (pad 184)
VSLOT = NCOL * 128            # 125184
NT = VSLOT // 256             # 489 matmul tiles (256 voxels each)
GROUP_T = 8                   # tiles per PSUM bank group
CHUNK_T = 64                  # tiles per ft DMA chunk

_prog_cache = {}


def _build_program(use_b1: bool, loop_iters: int = 1):
    nc = bacc.Bacc("TRN2", target_bir_lowering=False, debug=False)
    ft = nc.dram_tensor("ft", [128, NT * 128], mybir.dt.float32, kind="ExternalInput")
    wstack = nc.dram_tensor("wstack", [128, 64], mybir.dt.float32, kind="ExternalInput")
    w2rep = nc.dram_tensor("w2rep", [128, 512], mybir.dt.float32, kind="ExternalInput")
    b2rep = nc.dram_tensor("b2rep", [128, 1], mybir.dt.float32, kind="ExternalInput")
    if use_b1:
        b1stack = nc.dram_tensor("b1stack", [1, 64], mybir.dt.float32, kind="ExternalInput")
    score_o = nc.dram_tensor("score_o", [128, NCOL], mybir.dt.float32, kind="ExternalOutput")

    with tile.TileContext(nc) as tc:
        with (
            tc.tile_pool(name="consts", bufs=1) as cpool,
            tc.tile_pool(name="ftchunks", bufs=3) as fpool,
            tc.tile_pool(name="psum", bufs=2, space="PSUM") as ppool,
            tc.tile_pool(name="work", bufs=2) as wpool,
            tc.tile_pool(name="persist", bufs=1) as lpool,
        ):
            def emit_body():
                _emit(nc, tc, cpool, fpool, ppool, wpool, lpool,
                      ft, wstack, w2rep, b2rep,
                      b1stack if use_b1 else None, score_o, use_b1)

            if loop_iters > 1:
                with tc.For_i(0, loop_iters, 1):
                    emit_body()
            else:
                emit_body()
    nc.compile()
    return nc


def _emit(nc, tc, cpool, fpool, ppool, wpool, lpool,
          ft, wstack, w2rep, b2rep, b1stack, score_o, use_b1):
    if True:
        if True:
            wstack_s = cpool.tile([128, 64], mybir.dt.float32)
            nc.sync.dma_start(out=wstack_s[:], in_=wstack.ap())
            w2rep_s = cpool.tile([128, 512], mybir.dt.float32)
            nc.sync.dma_start(out=w2rep_s[:], in_=w2rep.ap())
            b2rep_s = cpool.tile([128, 1], mybir.dt.float32)
            nc.sync.dma_start(out=b2rep_s[:], in_=b2rep.ap())
            if use_b1:
                b1stack_s = cpool.tile([1, 64], mybir.dt.float32)
                nc.sync.dma_start(out=b1stack_s[:], in_=b1stack.ap())
                ones_s = cpool.tile([1, 128], mybir.dt.float32)
                nc.vector.memset(ones_s[:], 1.0)

            logit = lpool.tile([128, NCOL], mybir.dt.float32)
            score_t = lpool.tile([128, NCOL], mybir.dt.float32)

            n_chunks = (NT + CHUNK_T - 1) // CHUNK_T
            chunk_tiles = []
            for ci in range(n_chunks):
                t0 = ci * CHUNK_T
                t1 = min(t0 + CHUNK_T, NT)
                nt = t1 - t0
                wtg = fpool.tile([128, CHUNK_T * 128], mybir.dt.float32, name="wtg", tag="wtg")
                nc.sync.dma_start(
                    out=wtg[:, : nt * 128], in_=ft.ap()[:, t0 * 128 : t1 * 128]
                )
                chunk_tiles.append((wtg, t0, t1))

            # group loop: 8 tiles -> one PSUM bank [128, 512]
            g = 0
            t = 0
            while t < NT:
                gt0 = t
                gt1 = min(t + GROUP_T, NT)
                ng = gt1 - gt0
                w = ng * 64
                ps = ppool.tile([128, 512], mybir.dt.float32, name="ps")
                for i in range(ng):
                    tt = gt0 + i
                    ci = tt // CHUNK_T
                    wtg, ct0, _ = chunk_tiles[ci]
                    lhsT = wtg[:, (tt - ct0) * 128 : (tt - ct0 + 1) * 128]
                    if use_b1:
                        nc.tensor.matmul(
                            out=ps[:, i * 64 : (i + 1) * 64],
                            lhsT=ones_s[:],
                            rhs=b1stack_s[:],
                            start=True,
                            stop=False,
                        )
                        nc.tensor.matmul(
                            out=ps[:, i * 64 : (i + 1) * 64],
                            lhsT=lhsT,
                            rhs=wstack_s[:],
                            start=False,
                            stop=True,
                        )
                    else:
                        nc.tensor.matmul(
                            out=ps[:, i * 64 : (i + 1) * 64],
                            lhsT=lhsT,
                            rhs=wstack_s[:],
                            start=True,
                            stop=True,
                        )
                ht = wpool.tile([128, 512], mybir.dt.float32, name="ht")
                nc.scalar.activation(
                    ht[:, :w], ps[:, :w], mybir.ActivationFunctionType.Relu
                )
                prod = wpool.tile([128, 512], mybir.dt.float32, name="prod")
                nc.vector.tensor_tensor(
                    out=prod[:, :w], in0=ht[:, :w], in1=w2rep_s[:, :w],
                    op=mybir.AluOpType.mult,
                )
                nc.vector.tensor_reduce(
                    out=logit[:, g * 16 : g * 16 + 2 * ng],
                    in_=prod[:, :w].rearrange("p (q j) -> p q j", j=32),
                    axis=mybir.AxisListType.X,
                    op=mybir.AluOpType.add,
                )
                g += 1
                t = gt1

            nc.scalar.activation(
                score_t[:], logit[:], mybir.ActivationFunctionType.Sigmoid,
                bias=b2rep_s[:, :1], scale=1.0,
            )
            nc.sync.dma_start(out=score_o.ap(), in_=score_t[:])


def get_program(use_b1: bool, loop_iters: int = 1):
    key = (bool(use_b1), loop_iters)
    if key not in _prog_cache:
        _prog_cache[key] = _build_program(bool(use_b1), loop_iters)
    return _prog_cache[key]


def build_in_maps(feats, W1, b1, W2, b2):
    """Per-core input maps. feats [N, 64] f32.

    ft layout per core: [128, NT*128] where
    ft2[two*64 + k, t*128 + v] = feats[base + t*256 + two*128 + v, k]."""
    wstack = np.zeros((128, 64), np.float32)
    wstack[0:64, 0:32] = W1
    wstack[64:128, 32:64] = W1
    w2rep = np.tile(np.ascontiguousarray(W2[:, 0]), (128, 16)).astype(np.float32)
    b2rep = np.full((128, 1), b2[0], np.float32)
    use_b1 = bool(np.any(b1 != 0))
    b1stack = np.concatenate([b1, b1]).astype(np.float32)[None, :] if use_b1 else None

    in_maps = []
    for k in range(NCORES):
        a = k * NPER
        bnd = a + VSLOT
        if bnd <= N:
            fv = feats[a:bnd]
        else:
            fv = np.zeros((VSLOT, F_IN), np.float32)
            fv[: N - a] = feats[a:]
        # [NT, 2, 128, 64] -> [2, 64, NT, 128] -> [128, NT*128]
        ftk = np.ascontiguousarray(
            fv.reshape(NT, 2, 128, F_IN).transpose(1, 3, 0, 2)
        ).reshape(128, NT * 128)
        m = {"ft": ftk, "wstack": wstack, "w2rep": w2rep, "b2rep": b2rep}
        if use_b1:
            m["b1stack"] = b1stack
        in_maps.append(m)
    return in_maps, use_b1


def run_device(feats, W1, b1, W2, b2):
    in_maps, use_b1 = build_in_maps(feats, W1, b1, W2, b2)
    nc = get_program(use_b1)
    res = bass_utils.run_bass_kernel_spmd(nc, in_maps, core_ids=list(range(NCORES)))
    s = np.empty(N, np.float32)
    for k in range(NCORES):
        flat = res.results[k]["score_o"].T.ravel()  # slot order v = col*128 + p
        take = min(VSLOT, N - k * NPER)
        s[k * NPER : k * NPER + take] = flat[:take]
    return s


# ---------------- host-side NMS / top-k ----------------

_OFF27 = None


def _off27():
    global _OFF27
    if _OFF27 is None:
        r = np.array([-1, 0, 1], np.int64)
        dx, dy, dz = np.meshgrid(r, r, r, indexing="ij")
        _OFF27 = ((dx.ravel() * (Y + 2) + dy.ravel()) * (Z + 2) + dz.ravel()).astype(
            np.int64
        )
    return _OFF27


def _peaks_for(cand_ids, key, order, skey, gridvals_sorted, s):
    """Exact peak flags for voxel ids cand_ids.

    gridvals_sorted: candidate-gated scores (s if s>SCORE_T else -inf) in
    `order` (sorted-by-key) order. Returns bool array aligned to cand_ids."""
    if len(cand_ids) == 0:
        return np.zeros(0, bool)
    nk = key[cand_ids][:, None] + _off27()[None, :]  # [K, 27]
    lo = np.searchsorted(skey, nk.ravel(), side="left")
    hi = np.searchsorted(skey, nk.ravel(), side="right")
    cnt = hi - lo
    tot = int(cnt.sum())
    K = len(cand_ids)
    hmax = np.full(K, -np.inf, np.float64)
    if tot > 0:
        idx = np.repeat(lo, cnt) + (
            np.arange(tot) - np.repeat(np.cumsum(cnt) - cnt, cnt)
        )
        vals = gridvals_sorted[idx]
        owner = np.repeat(np.arange(K * 27) // 27, cnt)
        np.maximum.at(hmax, owner, vals)
    sc = s[cand_ids].astype(np.float64)
    # peak: cand & (hmax == s) & (s > CENT_T).  hmax >= s always when cand,
    # so hmax <= s is equivalent to equality; for non-cand it can't pass
    # CENT_T anyway but gate explicitly for exactness.
    cand = s[cand_ids] > SCORE_T
    return cand & (hmax <= sc) & (s[cand_ids] > CENT_T)


def host_postprocess(s_raw, cb, cx, cy, cz, mask):
    s = np.where(mask, s_raw, np.float32(0.0)).astype(np.float32)

    key = (
        (cb.astype(np.int64) * (X + 2) + cx + 1) * (Y + 2) + cy + 1
    ) * (Z + 2) + cz + 1
    order = np.argsort(key, kind="stable")
    skey = key[order]
    gated = np.where(s > SCORE_T, s.astype(np.float64), -np.inf)
    gvs = gated[order]

    chosen = None
    K = 4096
    while True:
        if K >= N:
            cand_ids = np.arange(N)
            certified = True
            s_out = np.float32(-np.inf)
        else:
            part = np.argpartition(s, N - K)
            cand_ids = part[N - K :]
            s_out = s[part[: N - K]].max() if N - K > 0 else np.float32(-np.inf)
            certified = False
        pk = _peaks_for(cand_ids, key, order, skey, gvs, s)
        pids = cand_ids[pk]
        if len(pids) >= MAX_PEAKS:
            o = np.lexsort((pids, -s[pids].astype(np.float64)))
            top = pids[o[:MAX_PEAKS]]
            kth = s[top[-1]]
            if K >= N or kth > s_out:
                chosen = top
                break
        elif K >= N:
            # fewer than 128 peaks in the whole volume: fill with lowest
            # non-peak indices, scores -1.0 (top_k of where(peak, s, -1)).
            o = np.lexsort((pids, -s[pids].astype(np.float64)))
            ordered = pids[o]
            need = MAX_PEAKS - len(ordered)
            fill = np.setdiff1d(
                np.arange(MAX_PEAKS + len(ordered)), ordered, assume_unique=False
            )[:need]
            chosen = np.concatenate([ordered, fill])
            peak_scores = np.concatenate(
                [s[ordered], np.full(need, -1.0, np.float32)]
            ).astype(np.float32)
            coords = np.stack([cb, cx, cy, cz], axis=1).astype(np.int32)
            return (
                s[:, None].astype(np.float32),
                chosen.astype(np.int32),
                peak_scores,
                coords[chosen],
            )
        K *= 16

    peak_scores = s[chosen].astype(np.float32)
    coords = np.stack([cb, cx, cy, cz], axis=1).astype(np.int32)
    return (
        s[:, None].astype(np.float32),
        chosen.astype(np.int32),
        peak_scores,
        coords[chosen],
    )


def kernel(feats, cb, cx, cy, cz, mask, W1, b1, W2, b2):
    feats = np.asarray(feats, np.float32)
    cb = np.asarray(cb, np.int32)
    cx = np.asarray(cx, np.int32)
    cy = np.asarray(cy, np.int32)
    cz = np.asarray(cz, np.int32)
    mask = np.asarray(mask, bool)
    W1 = np.asarray(W1, np.float32)
    b1 = np.asarray(b1, np.float32)
    W2 = np.asarray(W2, np.float32)
    b2 = np.asarray(b2, np.float32)

    s_raw = run_device(feats, W1, b1, W2, b2)
    return host_postprocess(s_raw, cb, cx, cy, cz, mask)


# revision 6
# speedup vs baseline: 26.9350x; 26.9350x over previous
"""CentroidHead kernel for 8x Trainium2 NeuronCores (Bass/Tile, SPMD).

Device (8 cores, data-parallel over voxels): the FeatDecoder MLP
  h = relu(feats @ W1 + b1); logit = h @ W2; score = sigmoid(logit + b2)
which is the memory/compute-bound part (256 MB feats, ~4.2 GFLOP).

Host: masking, candidate-restricted submanifold 3x3x3 NMS with a
certified top-K filter (any voxel outside the examined top-K set has a
strictly smaller score than the 128th selected peak, so the selection
is provably identical to the full computation), and top-k assembly with
jax.lax.top_k tie semantics (stable, lowest index first).

The matmul packs two 128-voxel chunks into one K=128 weight load
(lhsT = [feats_A^T ; feats_B^T], rhs = [[W1, 0], [0, W1]]), so the PE
processes 256 voxels per 128-column weight load.
"""

import sys

for _p in ("/opt/trn_rl_repo", "/root/.axon_site/_ro/trn_rl_repo"):
    if _p not in sys.path:
        sys.path.append(_p)

import numpy as np

import concourse.bass as bass
import concourse.mybir as mybir
import concourse.tile as tile
import concourse.bacc as bacc
from concourse import bass_utils

# ---- problem constants (hardcoded per spec) ----
N = 1_000_000
B, X, Y, Z = 2, 512, 512, 64
F_IN, H = 64, 32
SCORE_T = np.float32(0.1)
CENT_T = np.float32(0.2)
MAX_PEAKS = 128  # MAX_TREES * B
NCORES = 8

NPER = N // NCORES            # 125000 voxels per core
NCOL = 978                    # SBUF columns; NCOL*128 = 125184 slots (pad 184)
VSLOT = NCOL * 128            # 125184
NT = VSLOT // 256             # 489 matmul tiles (256 voxels each)
GROUP_T = 8                   # tiles per PSUM bank group
CHUNK_T = 32                  # tiles per ft DMA chunk

_prog_cache = {}


def _build_program(use_b1: bool, npos: int, loop_iters: int = 1):
    nc = bacc.Bacc("TRN2", target_bir_lowering=False, debug=False)
    ft = nc.dram_tensor("ft", [128, NT * 256], mybir.dt.float16, kind="ExternalInput")
    rh = nc.dram_tensor("rh", [128, 64], mybir.dt.float16, kind="ExternalInput")
    rl = nc.dram_tensor("rl", [128, 64], mybir.dt.float16, kind="ExternalInput")
    b2rep = nc.dram_tensor("b2rep", [128, 1], mybir.dt.float32, kind="ExternalInput")
    if use_b1:
        b1stack = nc.dram_tensor("b1stack", [2, 64], mybir.dt.float16, kind="ExternalInput")
    score_o = nc.dram_tensor("score_o", [128, NCOL], mybir.dt.float32, kind="ExternalOutput")

    with tile.TileContext(nc) as tc:
        with (
            tc.tile_pool(name="consts", bufs=1) as cpool,
            tc.tile_pool(name="ftchunks", bufs=3) as fpool,
            tc.tile_pool(name="psum", bufs=2, space="PSUM") as ppool,
            tc.tile_pool(name="work", bufs=2) as wpool,
            tc.tile_pool(name="persist", bufs=1) as lpool,
        ):
            def emit_body():
                _emit(nc, tc, cpool, fpool, ppool, wpool, lpool,
                      ft, rh, rl, b2rep,
                      b1stack if use_b1 else None, score_o, use_b1, npos)

            if loop_iters > 1:
                with tc.For_i(0, loop_iters, 1):
                    emit_body()
            else:
                emit_body()
    nc.compile()
    return nc


def _emit(nc, tc, cpool, fpool, ppool, wpool, lpool,
          ft, rh, rl, b2rep, b1stack, score_o, use_b1, npos):
    rh_s = cpool.tile([128, 64], mybir.dt.float16, name="rh_s", tag="rh")
    nc.sync.dma_start(out=rh_s[:], in_=rh.ap())
    rl_s = cpool.tile([128, 64], mybir.dt.float16, name="rl_s", tag="rl")
    nc.sync.dma_start(out=rl_s[:], in_=rl.ap())
    b2rep_s = cpool.tile([128, 1], mybir.dt.float32, name="b2rep_s", tag="b2")
    nc.sync.dma_start(out=b2rep_s[:], in_=b2rep.ap())
    if use_b1:
        b1stack_s = cpool.tile([2, 64], mybir.dt.float16, name="b1stack_s", tag="b1")
        nc.sync.dma_start(out=b1stack_s[:], in_=b1stack.ap())
        ones2_s = cpool.tile([2, 128], mybir.dt.float16, name="ones2_s", tag="o2")
        nc.vector.memset(ones2_s[:], 1.0)

    logit = lpool.tile([128, NCOL], mybir.dt.float32, name="logit", tag="logit")
    score_t = lpool.tile([128, NCOL], mybir.dt.float32, name="score_t", tag="score_t")

    n_chunks = (NT + CHUNK_T - 1) // CHUNK_T
    chunk_tiles = []
    for ci in range(n_chunks):
        t0 = ci * CHUNK_T
        t1 = min(t0 + CHUNK_T, NT)
        wtg = fpool.tile([128, CHUNK_T * 256], mybir.dt.float16, name="wtg", tag="wtg")
        nc.sync.dma_start(
            out=wtg[:, : (t1 - t0) * 256], in_=ft.ap()[:, t0 * 256 : t1 * 256]
        )
        chunk_tiles.append((wtg, t0, t1))

    nneg = 32 - npos
    g = 0
    t = 0
    while t < NT:
        gt0 = t
        gt1 = min(t + GROUP_T, NT)
        ng = gt1 - gt0
        w = ng * 64
        ps = ppool.tile([128, 512], mybir.dt.float32, name="ps", tag="ps")
        for i in range(ng):
            tt = gt0 + i
            ci = tt // CHUNK_T
            wtg, ct0, _ = chunk_tiles[ci]
            base = (tt - ct0) * 256
            hT = wtg[:, base : base + 128]
            lT = wtg[:, base + 128 : base + 256]
            out = ps[:, i * 64 : (i + 1) * 64]
            if use_b1:
                nc.tensor.matmul(out=out, lhsT=ones2_s[:], rhs=b1stack_s[:],
                                 start=True, stop=False)
            nc.tensor.matmul(out=out, lhsT=hT, rhs=rh_s[:],
                             start=not use_b1, stop=False)
            nc.tensor.matmul(out=out, lhsT=hT, rhs=rl_s[:], start=False, stop=False)
            nc.tensor.matmul(out=out, lhsT=lT, rhs=rh_s[:], start=False, stop=True)
        ht = wpool.tile([128, 512], mybir.dt.float32, name="ht", tag="ht")
        nc.scalar.activation(ht[:, :w], ps[:, :w], mybir.ActivationFunctionType.Relu)
        # logit = sum(pos cols of relu) - sum(neg cols of relu) per voxel
        ht3 = ht[:, :w].rearrange("p (q j) -> p q j", j=32)
        lsl = logit[:, g * 16 : g * 16 + 2 * ng]
        if npos == 0:
            tmp = wpool.tile([128, 16], mybir.dt.float32, name="tmpr", tag="tmpr")
            nc.vector.tensor_reduce(out=tmp[:, : 2 * ng], in_=ht3,
                                    axis=mybir.AxisListType.X, op=mybir.AluOpType.add)
            nc.vector.tensor_scalar_mul(lsl, tmp[:, : 2 * ng], -1.0)
        elif nneg == 0:
            nc.vector.tensor_reduce(out=lsl, in_=ht3,
                                    axis=mybir.AxisListType.X, op=mybir.AluOpType.add)
        else:
            tmp = wpool.tile([128, 16], mybir.dt.float32, name="tmpr", tag="tmpr")
            nc.vector.tensor_reduce(out=tmp[:, : 2 * ng], in_=ht3[:, :, :npos],
                                    axis=mybir.AxisListType.X, op=mybir.AluOpType.add)
            tmpn = wpool.tile([128, 16], mybir.dt.float32, name="tmpn", tag="tmpn")
            nc.vector.tensor_reduce(out=tmpn[:, : 2 * ng], in_=ht3[:, :, npos:],
                                    axis=mybir.AxisListType.X, op=mybir.AluOpType.add)
            nc.vector.tensor_tensor(out=lsl, in0=tmp[:, : 2 * ng],
                                    in1=tmpn[:, : 2 * ng],
                                    op=mybir.AluOpType.subtract)
        g += 1
        t = gt1

    nc.scalar.activation(
        score_t[:], logit[:], mybir.ActivationFunctionType.Sigmoid,
        bias=b2rep_s[:, :1], scale=1.0,
    )
    nc.sync.dma_start(out=score_o.ap(), in_=score_t[:])


def get_program(use_b1: bool, npos: int, loop_iters: int = 1):
    key = (bool(use_b1), int(npos), loop_iters)
    if key not in _prog_cache:
        _prog_cache[key] = _build_program(bool(use_b1), int(npos), loop_iters)
    return _prog_cache[key]


def build_in_maps(feats, W1, b1, W2, b2):
    """Per-core inputs.

    W2's magnitude is folded into W1 (Wp = W1 * |W2|) with columns sorted
    [W2>0 | W2<=0]; after relu the kernel sums pos cols minus neg cols.
    Everything ships as exact fp16 hi/lo splits so the PE runs at fp16 rate
    while reproducing fp32 products to ~2^-22.

    ft layout per core ([128, NT*256] f16): tile t covers voxels
    [t*256, (t+1)*256); partitions [2*64] = [feats_hi of chunk(two=0|1)];
    free cols [t*256 + half*128 + v] hold hi (half=0) / lo (half=1) of
    voxel v in chunk `two`... see code.
    """
    w2 = W2[:, 0].astype(np.float64)
    sgn_pos = w2 > 0
    npos = int(sgn_pos.sum())
    perm = np.argsort(~sgn_pos, kind="stable")  # pos cols first
    Wp = (W1.astype(np.float64) * np.abs(w2)[None, :]).astype(np.float32)[:, perm]
    Wh = Wp.astype(np.float16)
    Wl = (Wp - Wh.astype(np.float32)).astype(np.float16)
    rhm = np.zeros((128, 64), np.float16)
    rlm = np.zeros((128, 64), np.float16)
    rhm[0:64, 0:32] = Wh
    rhm[64:128, 32:64] = Wh
    rlm[0:64, 0:32] = Wl
    rlm[64:128, 32:64] = Wl
    b2rep = np.full((128, 1), b2[0], np.float32)
    use_b1 = bool(np.any(b1 != 0))
    if use_b1:
        b1p = (b1.astype(np.float64) * np.abs(w2)).astype(np.float32)[perm]
        b1h = b1p.astype(np.float16)
        b1l = (b1p - b1h.astype(np.float32)).astype(np.float16)
        b1stack = np.stack(
            [np.concatenate([b1h, b1h]), np.concatenate([b1l, b1l])]
        ).astype(np.float16)
    in_maps = []
    for k in range(NCORES):
        a = k * NPER
        bnd = a + VSLOT
        if bnd <= N:
            fv = feats[a:bnd]
        else:
            fv = np.zeros((VSLOT, F_IN), np.float32)
            fv[: N - a] = feats[a:]
        h16 = fv.astype(np.float16)
        l16 = (fv - h16.astype(np.float32)).astype(np.float16)
        # [NT, 2(two), 128, 64] per plane; stack hi/lo as `half` axis
        A = h16.reshape(NT, 2, 128, F_IN)
        Bm = l16.reshape(NT, 2, 128, F_IN)
        C = np.stack([A, Bm], axis=2)  # [NT, two, half, v, k]
        ftk = np.ascontiguousarray(
            C.transpose(1, 4, 0, 2, 3)  # [two, k, NT, half, v]
        ).reshape(128, NT * 256)
        m = {"ft": ftk, "rh": rhm, "rl": rlm, "b2rep": b2rep}
        if use_b1:
            m["b1stack"] = b1stack
        in_maps.append(m)
    return in_maps, use_b1, npos


def run_device(feats, W1, b1, W2, b2):
    in_maps, use_b1, npos = build_in_maps(feats, W1, b1, W2, b2)
    nc = get_program(use_b1, npos)
    res = bass_utils.run_bass_kernel_spmd(nc, in_maps, core_ids=list(range(NCORES)))
    s = np.empty(N, np.float32)
    for k in range(NCORES):
        flat = res.results[k]["score_o"].T.ravel()  # slot order v = col*128 + p
        take = min(VSLOT, N - k * NPER)
        s[k * NPER : k * NPER + take] = flat[:take]
    return s


# ---------------- host-side NMS / top-k ----------------

_OFF27 = None


def _off27():
    global _OFF27
    if _OFF27 is None:
        r = np.array([-1, 0, 1], np.int64)
        dx, dy, dz = np.meshgrid(r, r, r, indexing="ij")
        _OFF27 = ((dx.ravel() * (Y + 2) + dy.ravel()) * (Z + 2) + dz.ravel()).astype(
            np.int64
        )
    return _OFF27


def _peaks_for(cand_ids, key, order, skey, gridvals_sorted, s):
    """Exact peak flags for voxel ids cand_ids.

    gridvals_sorted: candidate-gated scores (s if s>SCORE_T else -inf) in
    `order` (sorted-by-key) order. Returns bool array aligned to cand_ids."""
    if len(cand_ids) == 0:
        return np.zeros(0, bool)
    nk = key[cand_ids][:, None] + _off27()[None, :]  # [K, 27]
    lo = np.searchsorted(skey, nk.ravel(), side="left")
    hi = np.searchsorted(skey, nk.ravel(), side="right")
    cnt = hi - lo
    tot = int(cnt.sum())
    K = len(cand_ids)
    hmax = np.full(K, -np.inf, np.float64)
    if tot > 0:
        idx = np.repeat(lo, cnt) + (
            np.arange(tot) - np.repeat(np.cumsum(cnt) - cnt, cnt)
        )
        vals = gridvals_sorted[idx]
        owner = np.repeat(np.arange(K * 27) // 27, cnt)
        np.maximum.at(hmax, owner, vals)
    sc = s[cand_ids].astype(np.float64)
    # peak: cand & (hmax == s) & (s > CENT_T).  hmax >= s always when cand,
    # so hmax <= s is equivalent to equality; for non-cand it can't pass
    # CENT_T anyway but gate explicitly for exactness.
    cand = s[cand_ids] > SCORE_T
    return cand & (hmax <= sc) & (s[cand_ids] > CENT_T)


def host_postprocess(s_raw, cb, cx, cy, cz, mask):
    s = np.where(mask, s_raw, np.float32(0.0)).astype(np.float32)

    key = (
        (cb.astype(np.int64) * (X + 2) + cx + 1) * (Y + 2) + cy + 1
    ) * (Z + 2) + cz + 1
    order = np.argsort(key, kind="stable")
    skey = key[order]
    gated = np.where(s > SCORE_T, s.astype(np.float64), -np.inf)
    gvs = gated[order]

    chosen = None
    K = 4096
    while True:
        if K >= N:
            cand_ids = np.arange(N)
            certified = True
            s_out = np.float32(-np.inf)
        else:
            part = np.argpartition(s, N - K)
            cand_ids = part[N - K :]
            s_out = s[part[: N - K]].max() if N - K > 0 else np.float32(-np.inf)
            certified = False
        pk = _peaks_for(cand_ids, key, order, skey, gvs, s)
        pids = cand_ids[pk]
        if len(pids) >= MAX_PEAKS:
            o = np.lexsort((pids, -s[pids].astype(np.float64)))
            top = pids[o[:MAX_PEAKS]]
            kth = s[top[-1]]
            if K >= N or kth > s_out:
                chosen = top
                break
        elif K >= N:
            # fewer than 128 peaks in the whole volume: fill with lowest
            # non-peak indices, scores -1.0 (top_k of where(peak, s, -1)).
            o = np.lexsort((pids, -s[pids].astype(np.float64)))
            ordered = pids[o]
            need = MAX_PEAKS - len(ordered)
            fill = np.setdiff1d(
                np.arange(MAX_PEAKS + len(ordered)), ordered, assume_unique=False
            )[:need]
            chosen = np.concatenate([ordered, fill])
            peak_scores = np.concatenate(
                [s[ordered], np.full(need, -1.0, np.float32)]
            ).astype(np.float32)
            coords = np.stack([cb, cx, cy, cz], axis=1).astype(np.int32)
            return (
                s[:, None].astype(np.float32),
                chosen.astype(np.int32),
                peak_scores,
                coords[chosen],
            )
        K *= 16

    peak_scores = s[chosen].astype(np.float32)
    coords = np.stack([cb, cx, cy, cz], axis=1).astype(np.int32)
    return (
        s[:, None].astype(np.float32),
        chosen.astype(np.int32),
        peak_scores,
        coords[chosen],
    )


def kernel(feats, cb, cx, cy, cz, mask, W1, b1, W2, b2):
    feats = np.asarray(feats, np.float32)
    cb = np.asarray(cb, np.int32)
    cx = np.asarray(cx, np.int32)
    cy = np.asarray(cy, np.int32)
    cz = np.asarray(cz, np.int32)
    mask = np.asarray(mask, bool)
    W1 = np.asarray(W1, np.float32)
    b1 = np.asarray(b1, np.float32)
    W2 = np.asarray(W2, np.float32)
    b2 = np.asarray(b2, np.float32)

    s_raw = run_device(feats, W1, b1, W2, b2)
    return host_postprocess(s_raw, cb, cx, cy, cz, mask)
